# revision 16
# baseline (speedup 1.0000x reference)
"""Distributed brute-force KNN (IndexFlatL2, K=3) + mean of gathered pred values.

Strategy (data-parallel over the memory bank N, queries replicated):
  - Host sorts the memory rows by ||m||^2 and shards the sorted bank across
    the 8 cores (12500 rows each), transposed so the PE moving operand
    [K=d, N=n] streams straight from DRAM.
  - Phase 1 (device): c[b, n] = (2q).m_n via fp8e4m3 DoubleRow matmuls
    (0.5 PE cycles/column, contraction pairs of k-subtiles) into fp32 PSUM.
    DVE tensor_reduce window-maxes each PSUM block (windows of WND=25
    columns); because rows are msq-sorted, ||m||^2 is nearly constant
    within a window, so the window's best score s' = 2q.m - ||m||^2 is
    upper-bounded by wmax(c) - min_msq_window (admissible even in the
    sorted tail where the within-window msq spread grows). The subtract
    runs on the otherwise-idle GpSimd engine. max8 + max_index over each
    100-window segment (NSEG=5 segments) run interleaved with the next
    group's matmuls, so no serial DVE tail remains; each core returns 40
    candidate windows per query.
  - Phase 2 (host): rank the 320 candidate windows per query, take the
    top WSEL, exactly re-score their rows (fp64), take the true top-3,
    gather pred_values (through the sort permutation), return the mean.
"""

import sys
import types

import ml_dtypes
import numpy as np

try:  # bass_utils' axon trace path imports this unconditionally when
    import antenv.axon_hooks  # noqa: F401  # BASS_TRACE is set; stub it if absent
except ImportError:
    # Provide a functional stand-in: drive NTFF profiling via ctypes on
    # the axon PJRT .so (same contract as trn_agent_boot's hook).
    import contextlib
    import ctypes

    def _make_ntff_hook():
        so = "/opt/axon/libaxon_pjrt.so"
        try:
            lib = ctypes.CDLL(so)
        except OSError:
            return None
        if not hasattr(lib, "axon_start_nrt_profile"):
            return None
        lib.axon_start_nrt_profile.argtypes = [
            ctypes.POINTER(ctypes.c_int64),
            ctypes.c_size_t,
        ]
        lib.axon_start_nrt_profile.restype = ctypes.c_int64
        lib.axon_stop_nrt_profile.argtypes = [ctypes.c_char_p]
        lib.axon_stop_nrt_profile.restype = ctypes.c_int64

        @contextlib.contextmanager
        def _hook(output_dir, device_ids):
            import jax

            jax.devices()
            if device_ids:
                ids = (ctypes.c_int64 * len(device_ids))(*device_ids)
                rc = lib.axon_start_nrt_profile(ids, len(device_ids))
            else:
                rc = lib.axon_start_nrt_profile(None, 0)
            if rc != 0:
                raise RuntimeError(f"axon_start_nrt_profile rc={rc}")
            try:
                yield
            finally:
                n = lib.axon_stop_nrt_profile(str(output_dir).encode())
                if n < 0:
                    raise RuntimeError(f"axon_stop_nrt_profile rc={n}")

        return _hook

    _ntff_hook = _make_ntff_hook()
    _stub = types.ModuleType("antenv.axon_hooks")
    _stub.get_axon_ntff_profile_hook = lambda: _ntff_hook
    _stub.set_axon_ntff_profile_hook = lambda hook: None
    sys.modules["antenv.axon_hooks"] = _stub

import concourse.bacc as bacc
import concourse.mybir as mybir
import concourse.tile as tile
from concourse import bass_utils

B = 1024            # queries
D = 1024            # embedding dim
N = 100000          # memory rows
NCORES = 8
NS = N // NCORES    # 12500 memory rows per core
BLK = 500           # matmul free-dim tile (fits one PSUM bank in fp32)
NBLK = NS // BLK    # 25 blocks per core
KT = D // 128       # 8 contraction tiles
BCH = B // 128      # 8 query chunks of 128
WND = 25            # window width for the DVE windowed max
NWIN = NS // WND    # 500 windows per core
WPB = BLK // WND    # 20 windows per block
TOPB = 8            # DVE max8 width
NSEG = 5            # window segments per core; top-8 windows per segment
SEGW = NWIN // NSEG  # 100 windows per segment
BLK_PER_SEG = NBLK // NSEG  # 5 blocks per segment
NCAND = NSEG * TOPB  # 40 candidate windows per query per core
K = 3
WSEL = 32           # windows exactly re-scored on host per query

# DMA group sizes (blocks per mov DMA): small leading groups cut the
# latency to the first matmul; 5-wide steady state keeps 2.5KB lines.
GROUPS = (1, 2, 4, 5, 5, 5, 3)
GROUP_W = max(GROUPS)

USE_FP8 = True      # False falls back to fp16 matmuls (k-step 1)

_CACHE = {}
LAST_RUN = None
LAST_TOP_IDX = None


def _build_program():
    nc = bacc.Bacc(
        "TRN2",
        target_bir_lowering=False,
        debug=False,
        enable_asserts=False,
        num_devices=NCORES,
    )
    f32 = mybir.dt.float32
    u32 = mybir.dt.uint32
    mmdt = mybir.dt.float8e4 if USE_FP8 else mybir.dt.float16
    kstep = 2 if USE_FP8 else 1
    perf_mode = mybir.MatmulPerfMode.DoubleRow if USE_FP8 else None
    ns = NBLK * BLK
    nwin = ns // WND
    b = BCH * 128

    u16 = mybir.dt.uint16
    mT = nc.dram_tensor("mT", [D, ns], mmdt, kind="ExternalInput").ap()
    qT = nc.dram_tensor("qT", [D, b], mmdt, kind="ExternalInput").ap()
    msqw = nc.dram_tensor("msqw", [1, nwin], f32, kind="ExternalInput").ap()
    # Outputs are partition-major [128, BCH*NCAND] so the final DMA is 128
    # contiguous 1280B lines instead of 1024 strided 160B lines (the host
    # un-permutes: query b = c*128 + p).
    out_vals = nc.dram_tensor(
        "out_vals", [128, BCH * NCAND], f32, kind="ExternalOutput"
    ).ap()
    out_idx = nc.dram_tensor(
        "out_idx", [128, BCH * NCAND], u16, kind="ExternalOutput"
    ).ap()

    mT_r = mT.rearrange("(o p) n -> p o n", p=128)
    qT_r = qT.rearrange("(o p) b -> p o b", p=128)
    ov_r = out_vals.rearrange("p (c j) -> p c j", c=BCH)
    oi_r = out_idx.rearrange("p (c j) -> p c j", c=BCH)

    groups = []
    g0 = 0
    for w in GROUPS:
        groups.append((g0, w))
        g0 += w
    assert g0 == NBLK

    with tile.TileContext(nc) as tc:
        with (
            tc.tile_pool(name="const", bufs=1) as cpool,
            tc.tile_pool(name="mov", bufs=2) as movpool,
            tc.tile_pool(name="psum", bufs=8, space="PSUM") as pspool,
        ):
            # Warm up the PE power state while the first DMAs are in flight.
            # The clock governor reacts to draw, not mere busyness (narrow
            # warm matmuls never left the low p-state), so issue a few
            # full-width DoubleRow matmuls on zeroed scratch — the same
            # intensity as the real ones — sized to end right when the first
            # real operands land. (The warm tile's PSUM bank recycles into
            # the matmul rotation.)
            wq = cpool.tile([128, kstep, BLK], mmdt, tag="warmq")
            wql = cpool.tile([128, kstep, 128], mmdt, tag="warmql")
            nc.gpsimd.memset(wq, 0)
            nc.gpsimd.memset(wql, 0)
            wp = pspool.tile([128, BLK], f32, tag="mm", name="warm_ps")
            for _ in range(6):
                nc.tensor.matmul(
                    wp,
                    lhsT=wql,
                    rhs=wq,
                    start=True,
                    stop=True,
                    perf_mode=perf_mode,
                )
            qt_sb = cpool.tile([128, KT, b], mmdt, tag="qt")
            msqw_bc = cpool.tile([128, nwin], f32, tag="msqwbc")
            wmax = cpool.tile([128, BCH, nwin], f32, tag="wmax")
            wsc = cpool.tile([128, BCH, nwin], f32, tag="wsc")
            cand_v = cpool.tile([128, BCH, NCAND], f32, tag="cv")
            cand_i = cpool.tile([128, BCH, NCAND], u16, tag="ci")

            # The first group's mov tile and qT are both split into k-pair
            # DMAs, interleaved in the order the first chunk's k-loop consumes
            # them, so the first matmul only waits for ~380KB instead of the
            # whole 1.5MB. msqw is only needed by the first gpsimd sub, well
            # into the run.
            mov0 = movpool.tile([128, KT, GROUP_W * BLK], mmdt, tag="mov")
            w0 = groups[0][1]
            for kp in range(0, KT, kstep):
                nc.sync.dma_start(
                    mov0[:, kp : kp + kstep, : w0 * BLK],
                    mT_r[:, kp : kp + kstep, : w0 * BLK],
                )
                nc.sync.dma_start(
                    qt_sb[:, kp : kp + kstep, :], qT_r[:, kp : kp + kstep, :]
                )
            nc.sync.dma_start(msqw_bc, msqw.to_broadcast([128, nwin]))

            blocks_done = 0
            seg_done = 0
            for gi, (blk0, w) in enumerate(groups):
                n0 = blk0 * BLK
                wn = w * BLK
                if gi == 0:
                    mov = mov0
                else:
                    mov = movpool.tile([128, KT, GROUP_W * BLK], mmdt, tag="mov")
                    nc.sync.dma_start(mov[:, :, :wn], mT_r[:, :, n0 : n0 + wn])
                blocks_done += w
                segs_ready = []
                while (seg_done + 1) * BLK_PER_SEG <= blocks_done:
                    segs_ready.append(seg_done)
                    seg_done += 1
                for bc in range(BCH):
                    psums = [
                        pspool.tile([128, BLK], f32, tag="mm", name="mm_ps")
                        for _ in range(w)
                    ]
                    for k in range(0, KT, kstep):
                        lhsT = qt_sb[:, k : k + kstep, bc * 128 : (bc + 1) * 128]
                        for j in range(w):
                            nc.tensor.matmul(
                                psums[j],
                                lhsT=lhsT,
                                rhs=mov[:, k : k + kstep, j * BLK : (j + 1) * BLK],
                                start=(k == 0),
                                stop=(k + kstep >= KT),
                                perf_mode=perf_mode,
                            )
                    for j in range(w):
                        blk = blk0 + j
                        nc.vector.tensor_reduce(
                            wmax[:, bc, blk * WPB : (blk + 1) * WPB],
                            psums[j].rearrange("p (w t) -> p w t", t=WND),
                            axis=mybir.AxisListType.X,
                            op=mybir.AluOpType.max,
                            opt_input=False,
                        )
                    # Candidate phase for every segment this group completes,
                    # emitted per chunk so the last chunk's selection is the
                    # only work left after the final matmul.
                    for s in segs_ready:
                        sl = slice(s * SEGW, (s + 1) * SEGW)
                        nc.gpsimd.tensor_sub(
                            wsc[:, bc, sl], wmax[:, bc, sl], msqw_bc[:, sl]
                        )
                        cv = cand_v[:, bc, s * TOPB : (s + 1) * TOPB]
                        nc.vector.max(out=cv, in_=wsc[:, bc, sl])
                        nc.vector.max_index(
                            out=cand_i[:, bc, s * TOPB : (s + 1) * TOPB],
                            in_max=cv,
                            in_values=wsc[:, bc, sl],
                        )
            assert seg_done == NSEG
            nc.sync.dma_start(ov_r, cand_v)
            nc.sync.dma_start(oi_r, cand_i)
    nc.compile()
    return nc


def kernel(h_query, memory_embeds, pred_values):
    global LAST_RUN, LAST_TOP_IDX
    q = np.ascontiguousarray(np.asarray(h_query, dtype=np.float32))
    m = np.ascontiguousarray(np.asarray(memory_embeds, dtype=np.float32))
    pv = np.asarray(pred_values, dtype=np.float32)

    msq_full = np.einsum("nd,nd->n", m, m)
    perm = np.argsort(msq_full, kind="stable")
    m_s = m[perm]                      # msq-sorted memory bank
    msq_s = msq_full[perm]

    mmdt_np = ml_dtypes.float8_e4m3 if USE_FP8 else np.float16
    qTs = (np.ascontiguousarray(q.T) * np.float32(2.0)).astype(mmdt_np)
    mTs = np.ascontiguousarray(m_s.T).astype(mmdt_np)
    # Window correction uses the window MIN of ||m||^2: the corrected window
    # score is then an upper bound on every row score in the window
    # (admissible), so sorted-tail windows with a large msq spread can only
    # gain clutter, never lose the true top-3.
    msqw_all = msq_s.reshape(N // WND, WND).min(axis=1).astype(np.float32)

    if "nc" not in _CACHE:
        _CACHE["nc"] = _build_program()
    nc = _CACHE["nc"]

    in_maps = []
    for c in range(NCORES):
        sl = slice(c * NS, (c + 1) * NS)
        wsl = slice(c * NWIN, (c + 1) * NWIN)
        in_maps.append(
            {
                "mT": np.ascontiguousarray(mTs[:, sl]),
                "qT": qTs,
                "msqw": np.ascontiguousarray(msqw_all[wsl]).reshape(1, NWIN),
            }
        )

    res = bass_utils.run_bass_kernel_spmd(nc, in_maps, core_ids=list(range(NCORES)))
    LAST_RUN = res
    results = res.results

    # windows: value [B, 40] + in-segment index [B, 40] per core; global
    # window id = core*NWIN + seg*SEGW + idx; window w covers sorted rows
    # [w*WND, +WND). Device emits [128, BCH*NCAND] partition-major; query
    # b = c*128 + p.
    def _unperm(a):
        return (
            a.reshape(128, BCH, NCAND).transpose(1, 0, 2).reshape(B, NCAND)
        )

    seg_off = (np.arange(NCAND, dtype=np.int64) // TOPB) * SEGW
    vals = np.concatenate([_unperm(r["out_vals"]) for r in results], axis=1)
    widx = np.concatenate(
        [
            _unperm(r["out_idx"]).astype(np.int64) + seg_off[None, :] + c * NWIN
            for c, r in enumerate(results)
        ],
        axis=1,
    )

    # Phase 2: pick top-WSEL windows per query, exactly re-score their rows
    # (chunked over queries to bound the fp64 gather's footprint).
    sel = np.argpartition(-vals, WSEL, axis=1)[:, :WSEL]
    wsel = np.take_along_axis(widx, sel, axis=1)           # [B, WSEL]
    rows = wsel[:, :, None] * WND + np.arange(WND)[None, None, :]
    cidx = rows.reshape(B, WSEL * WND)                     # sorted-space rows
    top_sorted = np.empty((B, K), dtype=np.int64)
    qf = q.astype(np.float64)
    CB = 256
    for i0 in range(0, B, CB):
        i1 = i0 + CB
        mg = m_s[cidx[i0:i1]].astype(np.float64)           # [CB, WSEL*WND, D]
        s_exact = 2.0 * np.einsum("bd,bkd->bk", qf[i0:i1], mg)
        s_exact -= np.einsum("bkd,bkd->bk", mg, mg)
        pick = np.argpartition(-s_exact, K, axis=1)[:, :K]
        top_sorted[i0:i1] = np.take_along_axis(cidx[i0:i1], pick, axis=1)
    top_idx = perm[top_sorted]                             # original row ids
    LAST_TOP_IDX = top_idx
    y = pv[top_idx].astype(np.float64).mean()
    return np.float32(y)


# revision 19
# speedup vs baseline: 1.0104x; 1.0104x over previous
"""Distributed brute-force KNN (IndexFlatL2, K=3) + mean of gathered pred values.

Strategy (data-parallel over the memory bank N, queries replicated):
  - Host sorts the memory rows by ||m||^2 and shards the sorted bank across
    the 8 cores (12500 rows each), transposed so the PE moving operand
    [K=d, N=n] streams straight from DRAM.
  - Phase 1 (device): c[b, n] = (2q).m_n via fp8e4m3 DoubleRow matmuls
    (0.5 PE cycles/column, contraction pairs of k-subtiles) into fp32 PSUM.
    DVE tensor_reduce window-maxes each PSUM block (windows of WND=25
    columns); because rows are msq-sorted, ||m||^2 is nearly constant
    within a window, so the window's best score s' = 2q.m - ||m||^2 is
    upper-bounded by wmax(c) - min_msq_window (admissible even in the
    sorted tail where the within-window msq spread grows). The subtract
    runs on the otherwise-idle GpSimd engine. max8 + max_index over each
    100-window segment (NSEG=5 segments) run interleaved with the next
    group's matmuls, so no serial DVE tail remains; each core returns 40
    candidate windows per query.
  - Phase 2 (host): rank the 320 candidate windows per query, take the
    top WSEL, exactly re-score their rows (fp64), take the true top-3,
    gather pred_values (through the sort permutation), return the mean.
"""

import sys
import types

import ml_dtypes
import numpy as np

try:  # bass_utils' axon trace path imports this unconditionally when
    import antenv.axon_hooks  # noqa: F401  # BASS_TRACE is set; stub it if absent
except ImportError:
    # Provide a functional stand-in: drive NTFF profiling via ctypes on
    # the axon PJRT .so (same contract as trn_agent_boot's hook).
    import contextlib
    import ctypes

    def _make_ntff_hook():
        so = "/opt/axon/libaxon_pjrt.so"
        try:
            lib = ctypes.CDLL(so)
        except OSError:
            return None
        if not hasattr(lib, "axon_start_nrt_profile"):
            return None
        lib.axon_start_nrt_profile.argtypes = [
            ctypes.POINTER(ctypes.c_int64),
            ctypes.c_size_t,
        ]
        lib.axon_start_nrt_profile.restype = ctypes.c_int64
        lib.axon_stop_nrt_profile.argtypes = [ctypes.c_char_p]
        lib.axon_stop_nrt_profile.restype = ctypes.c_int64

        @contextlib.contextmanager
        def _hook(output_dir, device_ids):
            import jax

            jax.devices()
            if device_ids:
                ids = (ctypes.c_int64 * len(device_ids))(*device_ids)
                rc = lib.axon_start_nrt_profile(ids, len(device_ids))
            else:
                rc = lib.axon_start_nrt_profile(None, 0)
            if rc != 0:
                raise RuntimeError(f"axon_start_nrt_profile rc={rc}")
            try:
                yield
            finally:
                n = lib.axon_stop_nrt_profile(str(output_dir).encode())
                if n < 0:
                    raise RuntimeError(f"axon_stop_nrt_profile rc={n}")

        return _hook

    _ntff_hook = _make_ntff_hook()
    _stub = types.ModuleType("antenv.axon_hooks")
    _stub.get_axon_ntff_profile_hook = lambda: _ntff_hook
    _stub.set_axon_ntff_profile_hook = lambda hook: None
    sys.modules["antenv.axon_hooks"] = _stub

import concourse.bacc as bacc
import concourse.mybir as mybir
import concourse.tile as tile
from concourse import bass_utils

B = 1024            # queries
D = 1024            # embedding dim
N = 100000          # memory rows
NCORES = 8
NS = N // NCORES    # 12500 memory rows per core
BLK = 500           # matmul free-dim tile (fits one PSUM bank in fp32)
NBLK = NS // BLK    # 25 blocks per core
KT = D // 128       # 8 contraction tiles
BCH = B // 128      # 8 query chunks of 128
WND = 25            # window width for the DVE windowed max
NWIN = NS // WND    # 500 windows per core
WPB = BLK // WND    # 20 windows per block
TOPB = 8            # DVE max8 width
NSEG = 5            # window segments per core; top-8 windows per segment
SEGW = NWIN // NSEG  # 100 windows per segment
BLK_PER_SEG = NBLK // NSEG  # 5 blocks per segment
NCAND = NSEG * TOPB  # 40 candidate windows per query per core
K = 3
WSEL = 32           # windows exactly re-scored on host per query

# DMA group sizes (blocks per mov DMA): small leading groups cut the
# latency to the first matmul; 5-wide steady state keeps 2.5KB lines.
GROUPS = (1, 2, 4, 5, 5, 5, 3)
GROUP_W = max(GROUPS)

USE_FP8 = True      # False falls back to fp16 matmuls (k-step 1)

_CACHE = {}
LAST_RUN = None
LAST_TOP_IDX = None


def _build_program():
    nc = bacc.Bacc(
        "TRN2",
        target_bir_lowering=False,
        debug=False,
        enable_asserts=False,
        num_devices=NCORES,
    )
    f32 = mybir.dt.float32
    u32 = mybir.dt.uint32
    mmdt = mybir.dt.float8e4 if USE_FP8 else mybir.dt.float16
    kstep = 2 if USE_FP8 else 1
    perf_mode = mybir.MatmulPerfMode.DoubleRow if USE_FP8 else None
    ns = NBLK * BLK
    nwin = ns // WND
    b = BCH * 128

    u16 = mybir.dt.uint16
    mT = nc.dram_tensor("mT", [D, ns], mmdt, kind="ExternalInput").ap()
    qT = nc.dram_tensor("qT", [D, b], mmdt, kind="ExternalInput").ap()
    msqw = nc.dram_tensor("msqw", [1, nwin], f32, kind="ExternalInput").ap()
    # Outputs are partition-major [128, BCH*NCAND] so the final DMA is 128
    # contiguous 1280B lines instead of 1024 strided 160B lines (the host
    # un-permutes: query b = c*128 + p).
    out_vals = nc.dram_tensor(
        "out_vals", [128, BCH * NCAND], f32, kind="ExternalOutput"
    ).ap()
    out_idx = nc.dram_tensor(
        "out_idx", [128, BCH * NCAND], u16, kind="ExternalOutput"
    ).ap()

    mT_r = mT.rearrange("(o p) n -> p o n", p=128)
    qT_r = qT.rearrange("(o p) b -> p o b", p=128)
    ov_r = out_vals.rearrange("p (c j) -> p c j", c=BCH)
    oi_r = out_idx.rearrange("p (c j) -> p c j", c=BCH)

    groups = []
    g0 = 0
    for w in GROUPS:
        groups.append((g0, w))
        g0 += w
    assert g0 == NBLK

    with tile.TileContext(nc) as tc:
        with (
            tc.tile_pool(name="const", bufs=1) as cpool,
            tc.tile_pool(name="mov", bufs=3) as movpool,
            tc.tile_pool(name="psum", bufs=8, space="PSUM") as pspool,
        ):
            # Warm up the PE power state while the first DMAs are in flight.
            # The clock governor reacts to draw, not mere busyness (narrow
            # warm matmuls never left the low p-state), so issue a few
            # full-width DoubleRow matmuls on zeroed scratch — the same
            # intensity as the real ones — sized to end right when the first
            # real operands land. (The warm tile's PSUM bank recycles into
            # the matmul rotation.)
            wq = cpool.tile([128, kstep, BLK], mmdt, tag="warmq")
            wql = cpool.tile([128, kstep, 128], mmdt, tag="warmql")
            nc.gpsimd.memset(wq, 0)
            nc.gpsimd.memset(wql, 0)
            wp = pspool.tile([128, BLK], f32, tag="mm", name="warm_ps")
            for _ in range(2):
                nc.tensor.matmul(
                    wp,
                    lhsT=wql,
                    rhs=wq,
                    start=True,
                    stop=True,
                    perf_mode=perf_mode,
                )
            qt_sb = cpool.tile([128, KT, b], mmdt, tag="qt")
            msqw_bc = cpool.tile([128, nwin], f32, tag="msqwbc")
            wmax = cpool.tile([128, BCH, nwin], f32, tag="wmax")
            wsc = cpool.tile([128, BCH, nwin], f32, tag="wsc")
            cand_v = cpool.tile([128, BCH, NCAND], f32, tag="cv")
            cand_i = cpool.tile([128, BCH, NCAND], u16, tag="ci")

            # Early DMA pipeline: at full PE speed the first three groups'
            # operands must land within ~30us, so their transfers are split
            # across parallel queues and issued ahead of the loop. Group 0
            # and qT interleave k-pair slices in the order the first chunk's
            # k-loop consumes them (first matmul waits on ~380KB only);
            # group 1 splits per block, group 2 in halves. msqw is only
            # needed by the first gpsimd sub, well into the run.
            pre_movs = {}
            w0 = groups[0][1]
            mov0 = movpool.tile([128, KT, GROUP_W * BLK], mmdt, tag="mov")
            pre_movs[0] = mov0
            for kp in range(0, KT, kstep):
                nc.sync.dma_start(
                    mov0[:, kp : kp + kstep, : w0 * BLK],
                    mT_r[:, kp : kp + kstep, : w0 * BLK],
                )
                nc.sync.dma_start(
                    qt_sb[:, kp : kp + kstep, :], qT_r[:, kp : kp + kstep, :]
                )
            for gi in (1, 2):
                blk0, w = groups[gi]
                n0 = blk0 * BLK
                mov = movpool.tile([128, KT, GROUP_W * BLK], mmdt, tag="mov")
                pre_movs[gi] = mov
                half = (w + 1) // 2
                for h0 in range(0, w, half):
                    hw = min(half, w - h0) * BLK
                    nc.sync.dma_start(
                        mov[:, :, h0 * BLK : h0 * BLK + hw],
                        mT_r[:, :, n0 + h0 * BLK : n0 + h0 * BLK + hw],
                    )
            nc.sync.dma_start(msqw_bc, msqw.to_broadcast([128, nwin]))

            blocks_done = 0
            seg_done = 0
            for gi, (blk0, w) in enumerate(groups):
                n0 = blk0 * BLK
                wn = w * BLK
                if gi in pre_movs:
                    mov = pre_movs[gi]
                else:
                    mov = movpool.tile([128, KT, GROUP_W * BLK], mmdt, tag="mov")
                    nc.sync.dma_start(mov[:, :, :wn], mT_r[:, :, n0 : n0 + wn])
                blocks_done += w
                segs_ready = []
                while (seg_done + 1) * BLK_PER_SEG <= blocks_done:
                    segs_ready.append(seg_done)
                    seg_done += 1
                for bc in range(BCH):
                    psums = [
                        pspool.tile([128, BLK], f32, tag="mm", name="mm_ps")
                        for _ in range(w)
                    ]
                    for k in range(0, KT, kstep):
                        lhsT = qt_sb[:, k : k + kstep, bc * 128 : (bc + 1) * 128]
                        for j in range(w):
                            nc.tensor.matmul(
                                psums[j],
                                lhsT=lhsT,
                                rhs=mov[:, k : k + kstep, j * BLK : (j + 1) * BLK],
                                start=(k == 0),
                                stop=(k + kstep >= KT),
                                perf_mode=perf_mode,
                            )
                    for j in range(w):
                        blk = blk0 + j
                        nc.vector.tensor_reduce(
                            wmax[:, bc, blk * WPB : (blk + 1) * WPB],
                            psums[j].rearrange("p (w t) -> p w t", t=WND),
                            axis=mybir.AxisListType.X,
                            op=mybir.AluOpType.max,
                            opt_input=False,
                        )
                    # Candidate phase for every segment this group completes,
                    # emitted per chunk so the last chunk's selection is the
                    # only work left after the final matmul.
                    for s in segs_ready:
                        sl = slice(s * SEGW, (s + 1) * SEGW)
                        nc.gpsimd.tensor_sub(
                            wsc[:, bc, sl], wmax[:, bc, sl], msqw_bc[:, sl]
                        )
                        cv = cand_v[:, bc, s * TOPB : (s + 1) * TOPB]
                        nc.vector.max(out=cv, in_=wsc[:, bc, sl])
                        nc.vector.max_index(
                            out=cand_i[:, bc, s * TOPB : (s + 1) * TOPB],
                            in_max=cv,
                            in_values=wsc[:, bc, sl],
                        )
            assert seg_done == NSEG
            nc.sync.dma_start(ov_r, cand_v)
            nc.sync.dma_start(oi_r, cand_i)
    nc.compile()
    return nc


def kernel(h_query, memory_embeds, pred_values):
    global LAST_RUN, LAST_TOP_IDX
    q = np.ascontiguousarray(np.asarray(h_query, dtype=np.float32))
    m = np.ascontiguousarray(np.asarray(memory_embeds, dtype=np.float32))
    pv = np.asarray(pred_values, dtype=np.float32)

    msq_full = np.einsum("nd,nd->n", m, m)
    perm = np.argsort(msq_full, kind="stable")
    m_s = m[perm]                      # msq-sorted memory bank
    msq_s = msq_full[perm]

    mmdt_np = ml_dtypes.float8_e4m3 if USE_FP8 else np.float16
    qTs = (np.ascontiguousarray(q.T) * np.float32(2.0)).astype(mmdt_np)
    mTs = np.ascontiguousarray(m_s.T).astype(mmdt_np)
    # Window correction uses the window MIN of ||m||^2: the corrected window
    # score is then an upper bound on every row score in the window
    # (admissible), so sorted-tail windows with a large msq spread can only
    # gain clutter, never lose the true top-3.
    msqw_all = msq_s.reshape(N // WND, WND).min(axis=1).astype(np.float32)

    if "nc" not in _CACHE:
        _CACHE["nc"] = _build_program()
    nc = _CACHE["nc"]

    in_maps = []
    for c in range(NCORES):
        sl = slice(c * NS, (c + 1) * NS)
        wsl = slice(c * NWIN, (c + 1) * NWIN)
        in_maps.append(
            {
                "mT": np.ascontiguousarray(mTs[:, sl]),
                "qT": qTs,
                "msqw": np.ascontiguousarray(msqw_all[wsl]).reshape(1, NWIN),
            }
        )

    res = bass_utils.run_bass_kernel_spmd(nc, in_maps, core_ids=list(range(NCORES)))
    LAST_RUN = res
    results = res.results

    # windows: value [B, 40] + in-segment index [B, 40] per core; global
    # window id = core*NWIN + seg*SEGW + idx; window w covers sorted rows
    # [w*WND, +WND). Device emits [128, BCH*NCAND] partition-major; query
    # b = c*128 + p.
    def _unperm(a):
        return (
            a.reshape(128, BCH, NCAND).transpose(1, 0, 2).reshape(B, NCAND)
        )

    seg_off = (np.arange(NCAND, dtype=np.int64) // TOPB) * SEGW
    vals = np.concatenate([_unperm(r["out_vals"]) for r in results], axis=1)
    widx = np.concatenate(
        [
            _unperm(r["out_idx"]).astype(np.int64) + seg_off[None, :] + c * NWIN
            for c, r in enumerate(results)
        ],
        axis=1,
    )

    # Phase 2: pick top-WSEL windows per query, exactly re-score their rows
    # (chunked over queries to bound the fp64 gather's footprint).
    sel = np.argpartition(-vals, WSEL, axis=1)[:, :WSEL]
    wsel = np.take_along_axis(widx, sel, axis=1)           # [B, WSEL]
    rows = wsel[:, :, None] * WND + np.arange(WND)[None, None, :]
    cidx = rows.reshape(B, WSEL * WND)                     # sorted-space rows
    top_sorted = np.empty((B, K), dtype=np.int64)
    qf = q.astype(np.float64)
    CB = 256
    for i0 in range(0, B, CB):
        i1 = i0 + CB
        mg = m_s[cidx[i0:i1]].astype(np.float64)           # [CB, WSEL*WND, D]
        s_exact = 2.0 * np.einsum("bd,bkd->bk", qf[i0:i1], mg)
        s_exact -= np.einsum("bkd,bkd->bk", mg, mg)
        pick = np.argpartition(-s_exact, K, axis=1)[:, :K]
        top_sorted[i0:i1] = np.take_along_axis(cidx[i0:i1], pick, axis=1)
    top_idx = perm[top_sorted]                             # original row ids
    LAST_TOP_IDX = top_idx
    y = pv[top_idx].astype(np.float64).mean()
    return np.float32(y)
